# revision 14
# baseline (speedup 1.0000x reference)
"""BiLSTM-CRF NLL kernel for Trainium2 (8 NeuronCores, data-parallel over batch).

Matches reference.py semantics:
  x = embedding[word_input]                       [B,S,E]
  x = BiLSTM0(x); x = BiLSTM1(x)                  [B,S,2H]
  x = LayerNorm(x) * g + b
  feats = x @ w_out.T + b_out                     [B,S,T]
  loss = mean_b( logZ(feats, trans) - gold_score )

Sharding: batch B=128 split across 8 cores (16 per core); all params
replicated.  mask is all-ones per the problem spec (fill: ones) and is
treated as such.

Device layout notes (per core, P=128 partitions):
  - tokens are indexed c = s*16 + b  (s in [0,256), b in [0,16))
  - x0T  [E, tok]   : embedding gathered then PE-transposed, bf16
  - xg   chunks     : input-projection pre-activations, produced
                      just-in-time in 32-step blocks, bf16,
                      free index = t_local*128 + m*16 + b  (m = 4H/128 tile)
  - hseq [128, S*32]: per (layer,dir) hidden states, bf16,
                      free index = s*32 + ht*16 + b (ht = H/128 tile)
  - recurrence: gates[4H,b] accumulate in one PSUM bank as 8 [128,16]
    regions; W_hh stationary (lhsT), h streamed (rhs), all bf16
  - CRF runs in exp-space: ea_t = (etrans.T @ ea_{t-1}) * exp(feats_t - DELTA)
    with etrans = exp(trans) stationary fp32; logZ recovered at the end.
"""

import numpy as np
from contextlib import ExitStack
from ml_dtypes import bfloat16

import concourse.bass as bass
import concourse.mybir as mybir
from concourse import tile, bacc
from concourse.bass_utils import run_bass_kernel_spmd
from concourse.masks import make_identity

F32 = mybir.dt.float32
BF16 = mybir.dt.bfloat16
F8E3 = mybir.dt.float8e3
I32 = mybir.dt.int32
AF = mybir.ActivationFunctionType
OP = mybir.AluOpType

WHH_FP8 = True   # W_hh stationary operand in fp8e3 (e3m4): halves LDWEIGHTS

V, E, H, TAG = 50000, 256, 256, 20
T = TAG + 2
START, STOP = TAG, TAG + 1
B, S_FULL = 128, 256
NCORES = 8
BL = B // NCORES            # 16 batch per core
P = 128
EPS = 1e-5
DELTA = 3.65                # exp-space shift per CRF step (~ln T + var(feats)/2)
RENORM = 64                 # CRF renormalization period (fp32-range insurance)
BLK = 32                    # steps per xg chunk


def build_program(S=S_FULL, repeat=1):
    NTOK = BL * S
    NG = NTOK // P          # gather tiles
    NCH = (NTOK + 511) // 512  # 512-col chunks over tokens
    nc = bacc.Bacc("TRN2", target_bir_lowering=False, debug=False)

    # ---------------- DRAM I/O ----------------
    WHH_DT = F8E3 if WHH_FP8 else BF16
    idx_d = nc.dram_tensor("idx", [P, NG], I32, kind="ExternalInput")
    embt_d = nc.dram_tensor("embt", [V, E], BF16, kind="ExternalInput")
    wih0_d = nc.dram_tensor("wih0t", [2, E, 4 * H], BF16, kind="ExternalInput")
    whh0_d = nc.dram_tensor("whh0t", [2, H, 4 * H], WHH_DT, kind="ExternalInput")
    bias0_d = nc.dram_tensor("bias0", [2, P, 8], F32, kind="ExternalInput")
    wih1_d = nc.dram_tensor("wih1t", [2, 2 * H, 4 * H], BF16, kind="ExternalInput")
    whh1_d = nc.dram_tensor("whh1t", [2, H, 4 * H], WHH_DT, kind="ExternalInput")
    bias1_d = nc.dram_tensor("bias1", [2, P, 8], F32, kind="ExternalInput")
    woutt_d = nc.dram_tensor("woutt", [2 * H, T], BF16, kind="ExternalInput")
    w1col_d = nc.dram_tensor("w1col", [T, 1], F32, kind="ExternalInput")
    boutcol_d = nc.dram_tensor("boutcol", [T, 1], F32, kind="ExternalInput")
    etrans_d = nc.dram_tensor("etrans", [T, T], F32, kind="ExternalInput")
    escol_d = nc.dram_tensor("escol", [T, 1], F32, kind="ExternalInput")
    eecol_d = nc.dram_tensor("eecol", [T, 1], F32, kind="ExternalInput")
    transbf_d = nc.dram_tensor("transbf", [T, T], BF16, kind="ExternalInput")
    tscol_d = nc.dram_tensor("tscol", [T, 1], BF16, kind="ExternalInput")
    tecol_d = nc.dram_tensor("tecol", [T, 1], BF16, kind="ExternalInput")
    tagsf_d = nc.dram_tensor("tagsf", [1, NTOK], F32, kind="ExternalInput")
    # one output row per repeat so no repeat body is dead-code-eliminable
    loss_d = nc.dram_tensor("loss", [repeat, BL], F32, kind="ExternalOutput")

    with tile.TileContext(nc) as tc:
      for _rep in range(repeat):
       with ExitStack() as top:
        cst = top.enter_context(tc.tile_pool(name="consts", bufs=1))

        # ---------- constants to SBUF ----------
        idx_sb = cst.tile([P, NG], I32)
        nc.sync.dma_start(idx_sb[:], idx_d[:])
        whh_sb = [[cst.tile([P, 2048], WHH_DT, name=f"whh{l}{d}") for d in range(2)]
                  for l in range(2)]
        for l, wd in ((0, whh0_d), (1, whh1_d)):
            for d in range(2):
                for k in range(2):
                    nc.sync.dma_start(whh_sb[l][d][:, k * 1024:(k + 1) * 1024],
                                      wd[d, k * P:(k + 1) * P, :])
        wih_sb = [[cst.tile([P, (2 + 2 * l) * 1024], BF16, name=f"wih{l}{d}")
                   for d in range(2)] for l in range(2)]
        for l, wd, nk in ((0, wih0_d, 2), (1, wih1_d, 4)):
            for d in range(2):
                for k in range(nk):
                    nc.sync.dma_start(wih_sb[l][d][:, k * 1024:(k + 1) * 1024],
                                      wd[d, k * P:(k + 1) * P, :])
        bias_sb = [cst.tile([P, 16], F32, name=f"bias{l}") for l in range(2)]
        for l, bd in ((0, bias0_d), (1, bias1_d)):
            for d in range(2):
                nc.sync.dma_start(bias_sb[l][:, d * 8:(d + 1) * 8], bd[d])
        woutt_sb = cst.tile([P, 4 * T], BF16)   # k-tile k at cols k*T
        for k in range(4):
            nc.sync.dma_start(woutt_sb[:, k * T:(k + 1) * T],
                              woutt_d[k * P:(k + 1) * P, :])
        w1col_sb = cst.tile([T, 1], F32)
        nc.sync.dma_start(w1col_sb[:], w1col_d[:])
        boutcol_sb = cst.tile([T, 1], F32)
        nc.sync.dma_start(boutcol_sb[:], boutcol_d[:])
        etrans_sb = cst.tile([T, T], F32)
        nc.sync.dma_start(etrans_sb[:], etrans_d[:])
        escol_sb = cst.tile([T, 1], F32)
        nc.sync.dma_start(escol_sb[:], escol_d[:])
        eecol_sb = cst.tile([T, 1], F32)
        nc.sync.dma_start(eecol_sb[:], eecol_d[:])
        transbf_sb = cst.tile([T, T], BF16)
        nc.sync.dma_start(transbf_sb[:], transbf_d[:])
        tscol_sb = cst.tile([T, 1], BF16)
        nc.sync.dma_start(tscol_sb[:], tscol_d[:])
        tecol_sb = cst.tile([T, 1], BF16)
        nc.sync.dma_start(tecol_sb[:], tecol_d[:])
        tagsf_sb = cst.tile([1, NTOK], F32)
        nc.sync.dma_start(tagsf_sb[:], tagsf_d[:])

        ident_sb = cst.tile([P, P], BF16)
        make_identity(nc, ident_sb[:])
        zeros_sb = cst.tile([P, 32], BF16)
        nc.vector.memset(zeros_sb[:], 0.0)
        ones128_sb = cst.tile([P, 1], BF16)
        nc.vector.memset(ones128_sb[:], 1.0)
        ones1x22_sb = cst.tile([1, T], F32)
        nc.vector.memset(ones1x22_sb[:], 1.0)
        ones22_sb = cst.tile([T, 1], BF16)
        nc.vector.memset(ones22_sb[:], 1.0)
        eps1 = cst.tile([1, 1], F32)
        nc.vector.memset(eps1[:], EPS)
        ndel22 = cst.tile([T, 1], F32)
        nc.vector.memset(ndel22[:], -DELTA)
        sdel1 = cst.tile([1, 1], F32)
        nc.vector.memset(sdel1[:], float(S) * DELTA)
        ident22f = cst.tile([T, T], F32)
        make_identity(nc, ident22f[:])
        iota_f = cst.tile([T, 1], F32)
        iota_i = cst.tile([T, 1], I32)
        nc.gpsimd.iota(iota_i[:], pattern=[[0, 1]], base=0, channel_multiplier=1)
        nc.vector.tensor_copy(iota_f[:], iota_i[:])

        # hidden-state sequences (bf16): free index = s*32 + ht*16 + b
        # hseq[1] must outlive the layer phase (consumed by LN/feats);
        # hseq[0] lives only through layer 1's input projection.
        hseq = [None,
                [cst.tile([P, S * 32], BF16, name=f"hseq1{d}") for d in range(2)]]

        # ---------- phase A: embedding gather + transpose ----------
        with ExitStack() as pa:
            h0pool = pa.enter_context(tc.tile_pool(name="hseq0p", bufs=1))
            hseq[0] = [h0pool.tile([P, S * 32], BF16, name=f"hseq0{d}")
                       for d in range(2)]
            x0t_pool = pa.enter_context(tc.tile_pool(name="x0t", bufs=1))
            x0t = x0t_pool.tile([P, 2 * NTOK], BF16)  # k*NTOK + c
            gpool = pa.enter_context(tc.tile_pool(name="gath", bufs=4))
            tpp = pa.enter_context(tc.tile_pool(name="tpsum", bufs=2, space="PSUM"))
            nblk = S // BLK
            gorder = []
            for g in range(NG):
                blk_of_g = (g * P) // (BLK * BL)  # which s-block this tile feeds
                gorder.append((0 if blk_of_g in (0, nblk - 1) else 1, g))
            for _, g in sorted(gorder):
                gt = gpool.tile([P, E], BF16, name="gt")
                nc.gpsimd.indirect_dma_start(
                    out=gt[:], out_offset=None, in_=embt_d[:],
                    in_offset=bass.IndirectOffsetOnAxis(ap=idx_sb[:, g:g + 1], axis=0))
                for k in range(2):
                    pt = tpp.tile([P, P], BF16, name="pt")
                    nc.tensor.transpose(out=pt[:], in_=gt[:, k * P:(k + 1) * P],
                                        identity=ident_sb[:])
                    nc.vector.tensor_copy(x0t[:, k * NTOK + g * P: k * NTOK + (g + 1) * P],
                                          pt[:])

            # ---------- the two BiLSTM layers ----------
            with ExitStack() as pl:
                projp = pl.enter_context(tc.tile_pool(name="projp", bufs=2, space="PSUM"))
                gatep = pl.enter_context(tc.tile_pool(name="gatep", bufs=2, space="PSUM"))
                xgpool = [pl.enter_context(tc.tile_pool(name=f"xg{d}", bufs=2))
                          for d in range(2)]
                spool = pl.enter_context(tc.tile_pool(name="srec", bufs=6))
                cpool = pl.enter_context(tc.tile_pool(name="cstate", bufs=1))
                c_sb = [cpool.tile([P, 32], F32, name=f"c{d}") for d in range(2)]

                def rhs_ktile(l, k, c0, n):
                    """[128, n-free] slice of layer-l input (K-tile k), tokens c0..c0+n."""
                    if l == 0:
                        return x0t[:, k * NTOK + c0: k * NTOK + c0 + n]
                    d, ht = divmod(k, 2)
                    hv = hseq[0][d][:].rearrange("p (s x) -> p s x", x=32)
                    s0, ns = c0 // BL, n // BL
                    return hv[:, s0:s0 + ns, ht * 16:ht * 16 + 16]

                def proj_thunk(l, d, m, blk, xg_tile):
                    def run():
                        nk = 2 + 2 * l
                        pp = projp.tile([P, BLK * BL], F32, name="pp")
                        for k in range(nk):
                            nc.tensor.matmul(
                                out=pp[:],
                                lhsT=wih_sb[l][d][:, k * 1024 + m * P: k * 1024 + (m + 1) * P],
                                rhs=rhs_ktile(l, k, blk * BLK * BL, BLK * BL),
                                start=(k == 0), stop=(k == nk - 1))
                        ov = xg_tile[:].rearrange("p (t x) -> p t x", x=P)
                        nc.scalar.activation(
                            ov[:, :, m * 16:(m + 1) * 16],
                            pp[:].rearrange("p (t b) -> p t b", b=BL),
                            AF.Identity,
                            bias=bias_sb[l][:, d * 8 + m: d * 8 + m + 1])
                    return run

                for l in range(2):
                    nblk = S // BLK
                    for d in range(2):
                        nc.vector.memset(c_sb[d][:], 0.0)
                    xg_cur = {}
                    pending = []

                    def emit_chunk(l, d, blk):
                        xg_t = xgpool[d].tile([P, BLK * P], BF16, name=f"xgt{d}")
                        xg_cur[(d, blk)] = xg_t
                        for m in range(8):
                            pending.append(proj_thunk(l, d, m, blk, xg_t))

                    # prologue: first chunk of each direction, run immediately
                    emit_chunk(l, 0, 0)
                    emit_chunk(l, 1, nblk - 1)
                    for th in pending:
                        th()
                    pending = []

                    for i in range(S):
                        j = i % BLK
                        if j == 0 and (i // BLK) + 1 < nblk:
                            emit_chunk(l, 0, i // BLK + 1)
                            emit_chunk(l, 1, nblk - 2 - i // BLK)
                        # spread pending proj work over the window
                        if pending:
                            nrun = max(1, (len(pending) + (BLK - 1 - j)) // (BLK - j))
                            for th in pending[:nrun]:
                                th()
                            pending = pending[nrun:]

                        # two independent per-direction chains; gate
                        # order is [g,i,f,o] (host-permuted); xg folded into
                        # PSUM via an identity matmul so ACT reads PSUM direct.
                        # ACT is split so the c-chain (needs g,i,f) starts
                        # after 12 of the 16 recurrent matmuls.
                        for d in range(2):
                            s = i if d == 0 else S - 1 - i
                            blk = s // BLK
                            tl = s % BLK
                            xg_t = xg_cur[(d, blk)]
                            gp = gatep.tile([P, P], F32, space="PSUM", name=f"gp{d}")
                            nc.tensor.matmul(
                                out=gp[:], lhsT=ident_sb[:],
                                rhs=xg_t[:, tl * P:(tl + 1) * P],
                                start=True, stop=False, skip_group_check=True)
                            for m in range(8):
                                for k in range(2):
                                    if i == 0:
                                        hp = zeros_sb[:, k * 16:(k + 1) * 16]
                                    else:
                                        sp = s - 1 if d == 0 else s + 1
                                        hp = hseq[l][d][:, sp * 32 + k * 16: sp * 32 + (k + 1) * 16]
                                    nc.tensor.matmul(
                                        out=gp[:, m * 16:(m + 1) * 16],
                                        lhsT=whh_sb[l][d][:, k * 1024 + m * P: k * 1024 + (m + 1) * P],
                                        rhs=hp, start=False, stop=(k == 1),
                                        skip_group_check=True)
                            sact = spool.tile([P, 128], F32, name=f"sact{d}")
                            nc.scalar.activation(sact[:, 0:32], gp[:, 0:32], AF.Tanh)
                            nc.scalar.activation(sact[:, 32:96], gp[:, 32:96],
                                                 AF.Sigmoid)
                            nc.scalar.activation(sact[:, 96:128], gp[:, 96:128],
                                                 AF.Sigmoid)
                            t2 = spool.tile([P, 32], F32, name=f"t2{d}")
                            nc.gpsimd.tensor_mul(t2[:], sact[:, 32:64], sact[:, 0:32])
                            t1 = spool.tile([P, 32], F32, name=f"t1{d}")
                            nc.vector.tensor_mul(t1[:], sact[:, 64:96], c_sb[d][:])
                            nc.vector.tensor_add(c_sb[d][:], t1[:], t2[:])
                            tcl = spool.tile([P, 32], F32, name=f"tc{d}")
                            nc.scalar.activation(tcl[:], c_sb[d][:], AF.Tanh)
                            nc.gpsimd.tensor_mul(
                                hseq[l][d][:, s * 32:(s + 1) * 32],
                                sact[:, 96:128], tcl[:])
                            # free consumed chunk bookkeeping
                            if tl == (BLK - 1 if d == 0 else 0):
                                xg_cur.pop((d, blk), None)

        # ---------- LayerNorm stats + feats + EE ----------
        post = top.enter_context(tc.tile_pool(name="post", bufs=1))
        feats_sb = post.tile([T, NTOK], F32)
        ee_sb = post.tile([T, NTOK], BF16)

        def h1k(k, c0, n):
            # layer-1 *output* K-tile k (d = k//2, ht = k%2), tokens c0..c0+n
            d, ht = divmod(k, 2)
            hv = hseq[1][d][:].rearrange("p (s x) -> p s x", x=32)
            s0, ns = c0 // BL, n // BL
            return hv[:, s0:s0 + ns, ht * 16:ht * 16 + 16]

        with ExitStack() as pf:
            statp = pf.enter_context(tc.tile_pool(name="statp", bufs=1, space="PSUM"))
            sqpool = pf.enter_context(tc.tile_pool(name="sqp", bufs=2))
            rows = pf.enter_context(tc.tile_pool(name="rows", bufs=1))
            fpsum = pf.enter_context(tc.tile_pool(name="fpsum", bufs=2, space="PSUM"))
            fsc = pf.enter_context(tc.tile_pool(name="fscratch", bufs=2))
            # pass 1: per-token LN stats for all chunks (all Sqrt ACTs batch
            # together — one table set, no Sqrt/Exp alternation)
            mus = rows.tile([1, NTOK], F32, name="mus")
            rstds = rows.tile([1, NTOK], F32, name="rstds")
            for c in range(NCH):
                c0 = c * 512
                sp = statp.tile([1, 512], F32, space="PSUM", name="sp")
                for k in range(4):
                    nc.tensor.matmul(out=sp[:], lhsT=ones128_sb[:],
                                     rhs=h1k(k, c0, 512), start=(k == 0),
                                     stop=(k == 3))
                sq = sqpool.tile([P, 4 * 512], BF16, name="sq")
                for k in range(4):
                    nc.scalar.activation(sq[:, k * 512:(k + 1) * 512],
                                         h1k(k, c0, 512), AF.Square)
                sp2 = statp.tile([1, 512], F32, space="PSUM", name="sp2")
                for k in range(4):
                    nc.tensor.matmul(out=sp2[:], lhsT=ones128_sb[:],
                                     rhs=sq[:, k * 512:(k + 1) * 512],
                                     start=(k == 0), stop=(k == 3))
                nc.scalar.mul(mus[:, c0:c0 + 512], sp[:], 1.0 / (2 * H))
                e2_row = sqpool.tile([1, 512], F32, name="e2_row")
                nc.scalar.mul(e2_row[:], sp2[:], 1.0 / (2 * H))
                m2_row = sqpool.tile([1, 512], F32, name="m2_row")
                nc.vector.tensor_mul(m2_row[:], mus[:, c0:c0 + 512],
                                     mus[:, c0:c0 + 512])
                var_row = sqpool.tile([1, 512], F32, name="var_row")
                nc.vector.tensor_sub(var_row[:], e2_row[:], m2_row[:])
                sd_row = sqpool.tile([1, 512], F32, name="sd_row")
                nc.scalar.activation(sd_row[:], var_row[:], AF.Sqrt, bias=eps1[:])
                nc.vector.reciprocal(rstds[:, c0:c0 + 512], sd_row[:])
            # pass 2: feats + exp for all chunks (all Exp ACTs together)
            for c in range(NCH):
                c0 = c * 512
                rawp = fpsum.tile([T, 512], F32, space="PSUM", name="rawp")
                for k in range(4):
                    nc.tensor.matmul(out=rawp[:], lhsT=woutt_sb[:, k * T:(k + 1) * T],
                                     rhs=h1k(k, c0, 512), start=(k == 0), stop=(k == 3))
                mup = fpsum.tile([T, 512], F32, space="PSUM", name="mup")
                nc.tensor.matmul(out=mup[:], lhsT=ones1x22_sb[:],
                                 rhs=mus[:, c0:c0 + 512], start=True, stop=True)
                rsp = fpsum.tile([T, 512], F32, space="PSUM", name="rsp")
                nc.tensor.matmul(out=rsp[:], lhsT=ones1x22_sb[:],
                                 rhs=rstds[:, c0:c0 + 512], start=True, stop=True)
                t1 = fsc.tile([T, 512], F32, name="ft1")
                nc.vector.tensor_scalar(t1[:], mup[:], w1col_sb[:], None, op0=OP.mult)
                t2 = fsc.tile([T, 512], F32, name="ft2")
                nc.vector.tensor_sub(t2[:], rawp[:], t1[:])
                t3 = fsc.tile([T, 512], F32, name="ft3")
                nc.vector.tensor_mul(t3[:], t2[:], rsp[:])
                nc.vector.tensor_scalar(feats_sb[:, c0:c0 + 512], t3[:], boutcol_sb[:],
                                        None, op0=OP.add)
                nc.scalar.activation(ee_sb[:, c0:c0 + 512], feats_sb[:, c0:c0 + 512],
                                     AF.Exp, bias=ndel22[:])

        # ---------- CRF forward in exp space ----------
        lz_row = post.tile([1, BL], F32)
        with ExitStack() as pc:
            eap = pc.enter_context(tc.tile_pool(name="eap", bufs=3))
            crfp = pc.enter_context(tc.tile_pool(name="crfp", bufs=2, space="PSUM"))
            rnp = pc.enter_context(tc.tile_pool(name="rnp", bufs=1, space="PSUM"))
            # two independent batch-halves pipeline PE (matmul) against DVE
            # (emission multiply)
            HB = BL // 2
            acc_row = eap.tile([1, BL], F32, name="acc_row")
            nc.vector.memset(acc_row[:], 0.0)
            eas = []
            for h in range(2):
                ea = eap.tile([T, HB], F32, name=f"ea{h}")
                nc.vector.tensor_scalar(ea[:], ee_sb[:, h * HB:(h + 1) * HB],
                                        escol_sb[:], None, op0=OP.mult)
                eas.append(ea)
            for t in range(1, S):
                pps = []
                for h in range(2):
                    pp = crfp.tile([T, HB], F32, space="PSUM", name=f"crfpp{h}")
                    nc.tensor.matmul(out=pp[:], lhsT=etrans_sb[:], rhs=eas[h][:],
                                     start=True, stop=True)
                    pps.append(pp)
                for h in range(2):
                    ea2 = eap.tile([T, HB], F32, name=f"ea{h}")
                    nc.vector.tensor_mul(
                        ea2[:], pps[h][:],
                        ee_sb[:, t * BL + h * HB: t * BL + (h + 1) * HB])
                    eas[h] = ea2
                if t % RENORM == 0 and t < S - 1:
                    # renormalize both halves: ea /= max_j(ea), acc += ln(max)
                    rmrow = eap.tile([1, BL], F32, name="rn_rmrow")
                    for h in range(2):
                        tp = rnp.tile([HB, T], F32, space="PSUM", name="rn_tp")
                        nc.tensor.transpose(out=tp[:], in_=eas[h][:],
                                            identity=ident22f[:])
                        m = eap.tile([HB, 1], F32, name="rn_m")
                        nc.vector.reduce_max(m[:], tp[:], axis=mybir.AxisListType.X)
                        rm = eap.tile([HB, 1], F32, name="rn_rm")
                        nc.vector.reciprocal(rm[:], m[:])
                        rp = rnp.tile([1, HB], F32, space="PSUM", name="rn_rp")
                        nc.tensor.transpose(out=rp[:], in_=rm[:],
                                            identity=ident22f[0:HB, 0:HB])
                        nc.vector.tensor_copy(rmrow[:, h * HB:(h + 1) * HB], rp[:])
                    lnr = eap.tile([1, BL], F32, name="rn_lnr")
                    nc.scalar.activation(lnr[:], rmrow[:], AF.Ln)
                    nc.vector.tensor_sub(acc_row[:], acc_row[:], lnr[:])
                    bp = rnp.tile([T, BL], F32, space="PSUM", name="rn_bp")
                    nc.tensor.matmul(out=bp[:], lhsT=ones1x22_sb[:], rhs=rmrow[:],
                                     start=True, stop=True)
                    for h in range(2):
                        ea3 = eap.tile([T, HB], F32, name=f"ea{h}")
                        nc.vector.tensor_mul(ea3[:], eas[h][:],
                                             bp[:, h * HB:(h + 1) * HB])
                        eas[h] = ea3
            zp = rnp.tile([1, BL], F32, space="PSUM", name="zp")
            for h in range(2):
                nc.tensor.matmul(out=zp[:, h * HB:(h + 1) * HB], lhsT=eecol_sb[:],
                                 rhs=eas[h][:], start=True, stop=True)
            nc.scalar.activation(lz_row[:], zp[:], AF.Ln)
            nc.vector.tensor_add(lz_row[:], lz_row[:], acc_row[:])

        # ---------- gold path score ----------
        out_row = post.tile([1, BL], F32)
        with ExitStack() as pg:
            gsc = pg.enter_context(tc.tile_pool(name="gsc", bufs=1))
            gch = pg.enter_context(tc.tile_pool(name="gch", bufs=2))
            gps = pg.enter_context(tc.tile_pool(name="gpsum", bufs=1, space="PSUM"))
            oh = gsc.tile([T, NTOK], BF16)
            for c in range(NCH):
                c0 = c * 512
                tbp = gps.tile([T, 512], F32, space="PSUM", name="tbp")
                nc.tensor.matmul(out=tbp[:], lhsT=ones1x22_sb[:],
                                 rhs=tagsf_sb[:, c0:c0 + 512], start=True, stop=True)
                nc.vector.tensor_scalar(oh[:, c0:c0 + 512], tbp[:], iota_f[:], None,
                                        op0=OP.is_equal)
            # emit score: sum_s feats[tag_c, c] accumulated per chunk
            emitS = gsc.tile([1, BL], F32)
            nc.vector.memset(emitS[:], 0.0)
            for c in range(NCH):
                c0 = c * 512
                m1 = gch.tile([T, 512], BF16, name="m1")
                nc.vector.tensor_mul(m1[:], feats_sb[:, c0:c0 + 512],
                                     oh[:, c0:c0 + 512])
                ep = gps.tile([1, 512], F32, space="PSUM", name="ep")
                nc.tensor.matmul(out=ep[:], lhsT=ones22_sb[:], rhs=m1[:],
                                 start=True, stop=True)
                part = gch.tile([1, BL], F32, name="part")
                nc.vector.reduce_sum(part[:],
                                     ep[:].rearrange("p (s b) -> p b s", b=BL),
                                     axis=mybir.AxisListType.X)
                nc.vector.tensor_add(emitS[:], emitS[:], part[:])
            # transition chain: G[j,c] = trans[tags_c, j] ; dot with onehot at c+BL
            NTR = NTOK - BL
            trS = gsc.tile([1, BL], F32)
            nc.vector.memset(trS[:], 0.0)
            c0 = 0
            while c0 < NTR:
                n = min(512, NTR - c0)
                gp2 = gps.tile([T, 512], F32, space="PSUM", name="gp2")
                nc.tensor.matmul(out=gp2[:, :n], lhsT=transbf_sb[:],
                                 rhs=oh[:, c0:c0 + n], start=True, stop=True)
                g2 = gch.tile([T, 512], BF16, name="g2")
                nc.scalar.copy(g2[:, :n], gp2[:, :n])
                m2 = gch.tile([T, 512], BF16, name="m2")
                nc.vector.tensor_mul(m2[:, :n], g2[:, :n],
                                     oh[:, c0 + BL:c0 + BL + n])
                tp2 = gps.tile([1, 512], F32, space="PSUM", name="tp2")
                nc.tensor.matmul(out=tp2[:, :n], lhsT=ones22_sb[:], rhs=m2[:, :n],
                                 start=True, stop=True)
                part2 = gch.tile([1, BL], F32, name="part2")
                nc.vector.reduce_sum(part2[:],
                                     tp2[:, :n].rearrange("p (s b) -> p b s", b=BL),
                                     axis=mybir.AxisListType.X)
                nc.vector.tensor_add(trS[:], trS[:], part2[:])
                c0 += n

            stp = gps.tile([1, BL], F32, space="PSUM", name="stp")
            nc.tensor.matmul(out=stp[:], lhsT=tscol_sb[:], rhs=oh[:, 0:BL],
                             start=True, stop=True)
            st0 = gsc.tile([1, BL], F32)
            nc.scalar.copy(st0[:], stp[:])
            sep = gps.tile([1, BL], F32, space="PSUM", name="sep")
            nc.tensor.matmul(out=sep[:], lhsT=tecol_sb[:], rhs=oh[:, NTOK - BL:NTOK],
                             start=True, stop=True)
            ste = gsc.tile([1, BL], F32)
            nc.scalar.copy(ste[:], sep[:])

            # loss = logZ + S*DELTA - emit - st0 - tr - ste
            a1 = gsc.tile([1, BL], F32)
            nc.scalar.activation(a1[:], lz_row[:], AF.Identity, bias=sdel1[:])
            a2 = gsc.tile([1, BL], F32)
            nc.vector.tensor_sub(a2[:], a1[:], emitS[:])
            a3 = gsc.tile([1, BL], F32)
            nc.vector.tensor_sub(a3[:], a2[:], st0[:])
            a4 = gsc.tile([1, BL], F32)
            nc.vector.tensor_sub(a4[:], a3[:], trS[:])
            nc.vector.tensor_sub(out_row[:], a4[:], ste[:])
        nc.sync.dma_start(loss_d[_rep:_rep + 1, :], out_row[:])

    nc.compile()
    return nc


GATE_PERM = np.r_[512:768, 0:256, 256:512, 768:1024]   # [i,f,g,o] -> [g,i,f,o]


def make_inmaps(inputs, S=S_FULL):
    """Host-side marshaling: slice batch, transpose/pack weights, cast dtypes."""
    f32 = np.float32
    emb = np.asarray(inputs["embedding"], f32).astype(bfloat16)
    word = np.asarray(inputs["word_input"]).astype(np.int32)[:, :S]
    tags = np.asarray(inputs["tags"]).astype(np.int32)[:, :S]
    trans = np.asarray(inputs["trans"], f32)
    ln_g = np.asarray(inputs["ln_g"], f32)
    ln_b = np.asarray(inputs["ln_b"], f32)
    w_out = np.asarray(inputs["w_out"], f32)
    b_out = np.asarray(inputs["b_out"], f32)

    wp = w_out * ln_g[None, :]                      # [T, 2H]
    boutp = b_out + w_out @ ln_b                    # [T]
    gp_ = GATE_PERM
    from ml_dtypes import float8_e3m4
    whh_dt = float8_e3m4 if WHH_FP8 else bfloat16
    shared = {
        "embt": emb,
        "wih0t": np.ascontiguousarray(
            np.asarray(inputs["w_ih0"], f32)[:, gp_, :].transpose(0, 2, 1)
        ).astype(bfloat16),
        "whh0t": np.ascontiguousarray(
            np.asarray(inputs["w_hh0"], f32)[:, gp_, :].transpose(0, 2, 1)
        ).astype(whh_dt),
        "bias0": np.ascontiguousarray(
            (np.asarray(inputs["b_ih0"], f32) + np.asarray(inputs["b_hh0"], f32))
            [:, gp_].reshape(2, 8, P).transpose(0, 2, 1)),
        "wih1t": np.ascontiguousarray(
            np.asarray(inputs["w_ih1"], f32)[:, gp_, :].transpose(0, 2, 1)
        ).astype(bfloat16),
        "whh1t": np.ascontiguousarray(
            np.asarray(inputs["w_hh1"], f32)[:, gp_, :].transpose(0, 2, 1)
        ).astype(whh_dt),
        "bias1": np.ascontiguousarray(
            (np.asarray(inputs["b_ih1"], f32) + np.asarray(inputs["b_hh1"], f32))
            [:, gp_].reshape(2, 8, P).transpose(0, 2, 1)),
        "woutt": np.ascontiguousarray(wp.T).astype(bfloat16),
        "w1col": np.ascontiguousarray(wp.sum(axis=1)[:, None]),
        "boutcol": np.ascontiguousarray(boutp[:, None]),
        "etrans": np.exp(trans),
        "escol": np.ascontiguousarray(np.exp(trans[START, :])[:, None]),
        "eecol": np.ascontiguousarray(np.exp(trans[:, STOP])[:, None]),
        "transbf": trans.astype(bfloat16),
        "tscol": np.ascontiguousarray(trans[START, :][:, None]).astype(bfloat16),
        "tecol": np.ascontiguousarray(trans[:, STOP][:, None]).astype(bfloat16),
    }
    NTOK = BL * S
    NG = NTOK // P
    in_maps = []
    for ci in range(NCORES):
        sl = slice(ci * BL, (ci + 1) * BL)
        w = word[sl]                               # [BL, S]
        arr = np.ascontiguousarray(w.T).reshape(-1)  # (s,b) order
        idx = np.ascontiguousarray(arr.reshape(NG, P).T)
        tg = np.ascontiguousarray(tags[sl].T).reshape(1, -1).astype(f32)
        m = dict(shared)
        m["idx"] = idx
        m["tagsf"] = tg
        in_maps.append(m)
    return in_maps


_NC_CACHE = {}


def kernel(**inputs) -> np.ndarray:
    if S_FULL not in _NC_CACHE:
        _NC_CACHE[S_FULL] = build_program(S_FULL)
    nc = _NC_CACHE[S_FULL]
    in_maps = make_inmaps(inputs, S_FULL)
    res = run_bass_kernel_spmd(nc, in_maps, core_ids=list(range(NCORES)))
    rows = np.concatenate([res.results[i]["loss"].reshape(-1)
                           for i in range(NCORES)])
    return np.float32(rows.mean())

